# revision 28
# baseline (speedup 1.0000x reference)
"""Trainium2 Bass kernel for RankGNN (2-layer GCN + head + mean-pool + pair diff).

Strategy (edge-parallel, graph-partitioned, per sharding hint):
 - Nodes are sharded across 8 cores at graph boundaries (batch is sorted), so
   pooling is fully core-local. Within a shard, nodes are permuted by degree
   (descending) so that fixed-height "transversal" edge tiles pack densely.
 - Message passing out[i] = dinv_i * sum_{j->i, j=i} xt_j with xt = dinv * x
   is computed by per-dst-block gathers: for a block of 128 dst nodes, slot p
   of every tile holds one in-edge of node p (or a zero row), so aggregation
   is a plain strided sum (VectorE reduce) -- no scatter needed.
 - Gathers use GPSIMD indirect DMA (per-edge row gather from DRAM), with the
   (A x) W reassociation so layer 1 gathers 16-float rows and layer 2 gathers
   32-float rows of g~ = (dinv*h1) @ W2.
 - Cross-core: AllGather of the xt / g~ node shards between layers.
 - Head MLP is folded: z = h2 @ (Wf1@Wf2) + (bf1@Wf2+bf2) (computed on device),
   pooling via one-hot matmul per block, pair diffs via tiny indirect gathers.
"""
import sys
import os

sys.path.insert(0, "/opt/trn_rl_repo")

import numpy as np

import concourse.bass as bass
import concourse.bacc as bacc
import concourse.tile as tile
import concourse.mybir as mybir
from concourse import bass_utils, library_config
from concourse.masks import make_identity

P = 128
F0P = 16          # padded input feature dim (9 -> 16)
F1 = 64
F2 = 32

N_CORES = 8
N_NODES = 100000
N_GRAPHS = 1024
N_PAIRS = 4096

CSB_MAX = 192     # max gather columns per superblock
CH = 16           # phase-1/phase-2 dma batching (blocks per chunk)


# ---------------------------------------------------------------------------
# Host-side preprocessing: sharding, degree-sorted blocking, transversal tiles
# ---------------------------------------------------------------------------

class Plan:
    pass


def make_plan(edge_index, batch, n_nodes, n_graphs, n_cores, n_pairs,
              idx_a, idx_b, csb_max=CSB_MAX):
    src = np.asarray(edge_index[0], dtype=np.int64)
    dst = np.asarray(edge_index[1], dtype=np.int64)
    batch = np.asarray(batch, dtype=np.int64)
    E = src.shape[0]

    pl = Plan()
    pl.n_cores = n_cores
    pl.gpc = n_graphs // n_cores          # graphs per core
    pl.ppc = n_pairs // n_cores           # pairs per core
    if pl.ppc >= P:
        assert pl.ppc % P == 0
        pl.p_use, pl.tp = P, pl.ppc // P
    else:
        pl.p_use, pl.tp = pl.ppc, 1

    # node ranges per core (graph-aligned)
    gstart = np.searchsorted(batch, np.arange(0, n_graphs + 1, pl.gpc))
    nodes_c = np.diff(gstart)
    NSH = int((int(nodes_c.max()) + 1 + P - 1) // P) * P
    NB = NSH // P
    pl.NSH, pl.NB = NSH, NB

    deg = np.bincount(dst, minlength=n_nodes).astype(np.int64) + 1  # + self loop
    dinv = (1.0 / np.sqrt(deg)).astype(np.float32)

    # per-core degree-desc permutation
    order_c = []
    gpos = np.empty(n_nodes, dtype=np.int64)
    for c in range(n_cores):
        ids = np.arange(gstart[c], gstart[c + 1])
        o = ids[np.argsort(-deg[ids], kind="stable")]
        order_c.append(o)
        gpos[o] = np.arange(len(o)) + c * NSH

    # source ranges: pairs of shards (int16-addressable: 2*NSH <= 32767)
    NR = (n_cores + 1) // 2
    RSZ = 2 * NSH
    assert RSZ <= 32767
    pl.NR, pl.RSZ = NR, RSZ
    src_gpos = gpos[src]
    rng_e = src_gpos // RSZ
    loc_e = (src_gpos - rng_e * RSZ).astype(np.int64)

    # in-edge CSR grouped by (dst, range); NO self-loops in edge lists
    key = dst * NR + rng_e
    eorder = np.argsort(key, kind="stable")
    loc_sorted = loc_e[eorder]
    cnt = np.bincount(key, minlength=n_nodes * NR).reshape(n_nodes, NR)
    off = np.zeros(n_nodes * NR + 1, dtype=np.int64)
    np.cumsum(cnt.reshape(-1), out=off[1:])
    off = off[:-1].reshape(n_nodes, NR)

    # pad (zero) row per range: first pad row of the even shard of that range
    pad_loc = np.array([len(order_c[2 * r]) for r in range(NR)], dtype=np.int64)

    # unified per-(block, range) tile counts T4 (max over cores)
    T4 = np.zeros((NB, NR), dtype=np.int64)
    for c in range(n_cores):
        o = order_c[c]
        for b in range(NB):
            rows = o[b * P:(b + 1) * P]
            if len(rows):
                T4[b] = np.maximum(T4[b], cnt[rows].max(axis=0))
    pl.T4 = T4
    pl.C_total = int(T4.sum())

    # superblocks: group blocks by total columns
    sbs = []  # list of dicts
    cur = []
    cur_cols = 0
    for b in range(NB):
        w = int(T4[b].sum())
        if cur and cur_cols + w > csb_max:
            sbs.append(cur)
            cur, cur_cols = [], 0
        cur.append(b)
        cur_cols += w
    if cur:
        sbs.append(cur)
    # per superblock: column layout r-major, then block, then t
    # col_in_sb[b][r] = start col of (b, r) within the sb's G tile
    sb_descs = []
    idxcol_cursor = 0
    for blks in sbs:
        col_in_sb = {}
        ccur = 0
        segs = []  # (r, g_col_start, ncols, idxcol_start)
        for r in range(NR):
            seg_start = ccur
            for b in blks:
                col_in_sb[(b, r)] = ccur
                ccur += int(T4[b][r])
            ncols = ccur - seg_start
            if ncols > 0:
                segs.append((r, seg_start, ncols, idxcol_cursor))
                idxcol_cursor += ncols * 8  # int16 idx cols (128 idx = 8 cols)
        sb_descs.append(dict(blks=blks, csb=ccur, col_in_sb=col_in_sb, segs=segs))
    pl.sb_descs = sb_descs
    pl.IDXC = idxcol_cursor

    # per-core data arrays
    pl.idx_all = []
    pl.x_shard = []
    pl.dinv_pb = []
    pl.batchl_pb = []
    pl.invcnt = []
    pl.gmask = []
    pl.pair_a = []
    pl.pair_b = []
    counts = np.bincount(batch, minlength=n_graphs).astype(np.float32)
    for c in range(n_cores):
        o = order_c[c]
        dinv_mat = np.zeros((P, NB), dtype=np.float32)
        batchl_mat = np.full((P, NB), -1.0, dtype=np.float32)
        # slot values per (block, range): [P, T4[b][r]]
        slot_vals = {}
        for b in range(NB):
            rows = o[b * P:(b + 1) * P]
            np_rows = len(rows)
            for r in range(NR):
                Tbr = int(T4[b][r])
                if Tbr == 0:
                    continue
                vals = np.full((P, Tbr), pad_loc[r], dtype=np.int64)
                if np_rows:
                    offs = off[rows, r][:, None] + np.arange(Tbr)[None, :]
                    valid = np.arange(Tbr)[None, :] < cnt[rows, r][:, None]
                    g = loc_sorted[np.minimum(offs, E - 1)]
                    vals[:np_rows] = np.where(valid, g, pad_loc[r])
                slot_vals[(b, r)] = vals
            if len(rows):
                dinv_mat[:len(rows), b] = dinv[rows]
                batchl_mat[:len(rows), b] = (batch[rows] - c * pl.gpc).astype(
                    np.float32)
        # assemble idx16 in wrapped per-chunk layout
        idx16 = np.zeros((P, pl.IDXC), dtype=np.int16)
        for sd in sb_descs:
            for (r, gc0, ncols, ic0) in sd["segs"]:
                # gather flat values in (col-major, partition-minor) order
                vcols = []
                for b in sd["blks"]:
                    if T4[b][r] > 0:
                        vcols.append(slot_vals[(b, r)])
                v = np.concatenate(vcols, axis=1)      # [P, ncols]
                assert v.shape[1] == ncols
                flat = v.T.reshape(-1)                  # i = col*128 + p
                w16 = flat.reshape(-1, 16).T            # [16, ncols*8]
                idx16[:, ic0:ic0 + ncols * 8] = np.tile(w16, (8, 1))
        pl.idx_all.append(idx16)
        xs = np.zeros((NSH, F0P), dtype=np.float32)
        pl.x_shard.append(xs)   # feature values filled in kernel() (needs x)
        pl.x_rows = getattr(pl, "x_rows", [])
        pl.dinv_pb.append(dinv_mat)
        pl.batchl_pb.append(batchl_mat)
        cc = counts[c * pl.gpc:(c + 1) * pl.gpc]
        iv = np.where(cc > 0, 1.0 / np.maximum(cc, 1.0), 0.0).astype(np.float32)
        pl.invcnt.append(iv[:, None])
        pl.gmask.append((cc > 0).astype(np.float32)[:, None])
        pa = np.zeros((pl.p_use, pl.tp), dtype=np.int32)
        pb = np.zeros((pl.p_use, pl.tp), dtype=np.int32)
        loc = np.asarray(idx_a[c * pl.ppc:(c + 1) * pl.ppc], dtype=np.int32)
        lob = np.asarray(idx_b[c * pl.ppc:(c + 1) * pl.ppc], dtype=np.int32)
        pa[:, :] = loc.reshape(pl.p_use, pl.tp)
        pb[:, :] = lob.reshape(pl.p_use, pl.tp)
        pl.pair_a.append(pa)
        pl.pair_b.append(pb)
    pl.order_c = order_c
    pl.n_graphs = n_graphs
    return pl


# ---------------------------------------------------------------------------
# Bass program builder (one SPMD program; per-core differences are data-only)
# ---------------------------------------------------------------------------

def build_program(pl, est=False):
    dt = mybir.dt
    NC = pl.n_cores
    NSH, NB = pl.NSH, pl.NB
    GPC = pl.gpc

    nc = bacc.Bacc("TRN2", target_bir_lowering=False, debug=False,
                   num_devices=1 if est else NC)

    def all_gather(in_ap, out_ap):
        if est:
            # cost-estimate single-core build: stand-in copy (collective time
            # is accounted separately)
            nc.sync.dma_start(out_ap[0:in_ap.shape[0]], in_ap)
        else:
            nc.gpsimd.collective_compute(
                "AllGather", mybir.AluOpType.bypass,
                replica_groups=[list(range(NC))],
                ins=[in_ap.opt()], outs=[out_ap.opt()])

    NR, RSZ = pl.NR, pl.RSZ
    # external inputs
    x_sh = nc.dram_tensor("x_shard", [NSH, F0P], dt.float32, kind="ExternalInput")
    idx_all = nc.dram_tensor("idx_all", [P, pl.IDXC], dt.int16, kind="ExternalInput")
    dinv_pb = nc.dram_tensor("dinv_pb", [P, NB], dt.float32, kind="ExternalInput")
    batchl_pb = nc.dram_tensor("batchl_pb", [P, NB], dt.float32, kind="ExternalInput")
    invcnt = nc.dram_tensor("invcnt", [GPC, 1], dt.float32, kind="ExternalInput")
    gmask = nc.dram_tensor("gmask", [GPC, 1], dt.float32, kind="ExternalInput")
    pair_a = nc.dram_tensor("pair_a", [pl.p_use, pl.tp], dt.int32, kind="ExternalInput")
    pair_b = nc.dram_tensor("pair_b", [pl.p_use, pl.tp], dt.int32, kind="ExternalInput")
    w1p = nc.dram_tensor("w1p", [F0P, F1], dt.float32, kind="ExternalInput")
    b1d = nc.dram_tensor("b1d", [1, F1], dt.float32, kind="ExternalInput")
    w2d = nc.dram_tensor("w2d", [F1, F2], dt.float32, kind="ExternalInput")
    b2d = nc.dram_tensor("b2d", [1, F2], dt.float32, kind="ExternalInput")
    wf1t = nc.dram_tensor("wf1t", [F2, F2], dt.float32, kind="ExternalInput")
    wf2d = nc.dram_tensor("wf2d", [F2, 1], dt.float32, kind="ExternalInput")
    bf1d = nc.dram_tensor("bf1d", [F2, 1], dt.float32, kind="ExternalInput")
    bf2d = nc.dram_tensor("bf2d", [1, 1], dt.float32, kind="ExternalInput")

    # external outputs
    xu_out = nc.dram_tensor("xu_out", [GPC, 1], dt.float32, kind="ExternalOutput")
    pairs_out = nc.dram_tensor("pairs_out", [pl.ppc, 1], dt.float32, kind="ExternalOutput")

    # internal dram (rows padded to 64 f32 = 256B for dma_gather)
    GW = 64
    xt_shard = nc.dram_tensor("xt_shard", [NSH, GW], dt.float32, kind="Internal")
    xt_full = nc.dram_tensor("xt_full", [NC * NSH, GW], dt.float32, kind="Internal",
                             addr_space="Shared")
    gt_shard = nc.dram_tensor("gt_shard", [NSH, GW], dt.float32, kind="Internal")
    gt_full = nc.dram_tensor("gt_full", [NC * NSH, GW], dt.float32, kind="Internal",
                             addr_space="Shared")
    util_sh = nc.dram_tensor("util_sh", [GPC, 1], dt.float32, kind="Internal")
    util_full = nc.dram_tensor("util_full", [NC * GPC, 1], dt.float32, kind="Internal",
                               addr_space="Shared")
    bh_dram = nc.dram_tensor("bh_dram", [1, 1], dt.float32, kind="Internal")

    groups = [list(range(NC))]

    with tile.TileContext(nc) as tc:
        with tc.tile_pool(name="const", bufs=1) as cp:
            nc.gpsimd.load_library(library_config.mlp)
            # constants in SBUF
            ident = cp.tile([P, P], dt.float32)
            make_identity(nc, ident[:])
            w1_s = cp.tile([F0P, F1], dt.float32)
            nc.sync.dma_start(w1_s[:], w1p[:])
            w2_s = cp.tile([F1, F2], dt.float32)
            nc.sync.dma_start(w2_s[:], w2d[:])
            wf1t_s = cp.tile([F2, F2], dt.float32)
            nc.sync.dma_start(wf1t_s[:], wf1t[:])
            wf2_s = cp.tile([F2, 1], dt.float32)
            nc.sync.dma_start(wf2_s[:], wf2d[:])
            bf1_s = cp.tile([F2, 1], dt.float32)
            nc.sync.dma_start(bf1_s[:], bf1d[:])
            bf2_s = cp.tile([1, 1], dt.float32)
            nc.sync.dma_start(bf2_s[:], bf2d[:])
            b1_bc = cp.tile([P, F1], dt.float32)
            nc.sync.dma_start(b1_bc[:], b1d[:].partition_broadcast(P))
            b2_bc = cp.tile([P, F2], dt.float32)
            nc.sync.dma_start(b2_bc[:], b2d[:].partition_broadcast(P))
            dinv_all = cp.tile([P, NB], dt.float32)
            nc.sync.dma_start(dinv_all[:], dinv_pb[:])
            batchl_all = cp.tile([P, NB], dt.float32)
            nc.sync.dma_start(batchl_all[:], batchl_pb[:])
            invcnt_s = cp.tile([GPC, 1], dt.float32)
            nc.sync.dma_start(invcnt_s[:], invcnt[:])
            gmask_s = cp.tile([GPC, 1], dt.float32)
            nc.sync.dma_start(gmask_s[:], gmask[:])
            iota_i = cp.tile([P, GPC], dt.int32)
            nc.gpsimd.iota(iota_i[:], pattern=[[1, GPC]], base=0, channel_multiplier=0)
            iota_f = cp.tile([P, GPC], dt.float32)
            nc.vector.tensor_copy(iota_f[:], iota_i[:])

            # folded head: w_head = Wf1 @ Wf2, b_head = bf1.Wf2 + bf2
            wh_s = cp.tile([F2, 1], dt.float32)
            bh_s = cp.tile([1, 1], dt.float32)
            with tc.tile_pool(name="cpsum", bufs=1, space="PSUM") as cpp:
                wh_p = cpp.tile([F2, 1], dt.float32, space="PSUM")
                nc.tensor.matmul(wh_p[:], lhsT=wf1t_s[:], rhs=wf2_s[:], start=True, stop=True)
                nc.vector.tensor_copy(wh_s[:], wh_p[:])
                bh_p = cpp.tile([1, 1], dt.float32, space="PSUM")
                nc.tensor.matmul(bh_p[:], lhsT=bf1_s[:], rhs=wf2_s[:], start=True, stop=True)
                nc.vector.tensor_tensor(bh_s[:], bh_p[:], bf2_s[:], op=mybir.AluOpType.add)
            nc.sync.dma_start(bh_dram[:], bh_s[:])
            bh_bc = cp.tile([GPC, 1], dt.float32)
            nc.sync.dma_start(bh_bc[:], bh_dram[:].partition_broadcast(GPC))

            util_acc = cp.tile([GPC, 1], dt.float32)
            nc.vector.memset(util_acc[:], 0.0)

            # ---------------- phase 1: xt = dinv * x (shard), AllGather ------
            with tc.tile_pool(name="p1", bufs=2) as p1:
                for j0 in range(0, NB, CH):
                    nch = min(CH, NB - j0)
                    xch = p1.tile([P, CH, F0P], dt.float32, tag="xch")
                    src_view = x_sh[j0 * P:(j0 + nch) * P, :].rearrange(
                        "(j p) f -> p j f", p=P)
                    nc.sync.dma_start(xch[:, :nch, :], src_view)
                    xs = p1.tile([P, CH, GW], dt.float32, tag="xs")
                    nc.vector.memset(xs[:], 0.0)
                    dv = dinv_all[:, j0:j0 + nch].unsqueeze(2).to_broadcast(
                        [P, nch, F0P])
                    nc.vector.tensor_tensor(xs[:, :nch, :F0P], xch[:, :nch, :], dv,
                                            op=mybir.AluOpType.mult)
                    dst_view = xt_shard[j0 * P:(j0 + nch) * P, :].rearrange(
                        "(j p) f -> p j f", p=P)
                    nc.sync.dma_start(dst_view, xs[:, :nch, :])
            all_gather(xt_shard[:], xt_full[:])

            # ---------------- phase 2: layer 1 + g~ ------------------------
            with tc.tile_pool(name="g1", bufs=2) as gp1, \
                 tc.tile_pool(name="ix1", bufs=2) as ixp, \
                 tc.tile_pool(name="blk1", bufs=3) as bp, \
                 tc.tile_pool(name="gs8", bufs=1) as gsp, \
                 tc.tile_pool(name="ps1", bufs=2, space="PSUM") as pp:
                gs_w = gsp.tile([P, NB, GW], dt.float32, tag="gsall")
                nc.vector.memset(gs_w[:], 0.0)
                for sd in pl.sb_descs:
                    csb = sd["csb"]
                    ic_lo = sd["segs"][0][3]
                    ic_hi = sd["segs"][-1][3] + sd["segs"][-1][2] * 8
                    idx_t = ixp.tile([P, CSB_MAX * 8], dt.int16, tag="idx")
                    nc.sync.dma_start(idx_t[:, :ic_hi - ic_lo],
                                      idx_all[:, ic_lo:ic_hi])
                    G = gp1.tile([P, CSB_MAX, GW], dt.float32, tag="G")
                    for (r, gc0, ncols, ic0) in sd["segs"]:
                        for k in range(0, ncols, 8):
                            nc_ = min(8, ncols - k)
                            nidx = nc_ * P
                            nc.gpsimd.dma_gather(
                                G[:, gc0 + k:gc0 + k + nc_, :],
                                xt_full[r * RSZ:(r + 1) * RSZ, :],
                                idx_t[:, ic0 - ic_lo + k * 8:
                                      ic0 - ic_lo + k * 8 + nc_ * 8],
                                nidx, nidx, GW)
                    for b in sd["blks"]:
                        dv = dinv_all[:, b:b + 1]
                        st = bp.tile([P, NR, F0P], dt.float32, tag="st")
                        nc.vector.memset(st[:], 0.0)
                        for r in range(NR):
                            Tbr = int(pl.T4[b][r])
                            if Tbr == 0:
                                continue
                            t0 = sd["col_in_sb"][(b, r)]
                            nc.vector.tensor_reduce(
                                st[:, r, :],
                                G[:, t0:t0 + Tbr, :F0P].transpose([0, 2, 1]),
                                axis=mybir.AxisListType.X, op=mybir.AluOpType.add)
                        accn = bp.tile([P, F0P], dt.float32, tag="accn")
                        nc.vector.tensor_reduce(
                            accn[:], st[:].transpose([0, 2, 1]),
                            axis=mybir.AxisListType.X, op=mybir.AluOpType.add)
                        xself = bp.tile([P, F0P], dt.float32, tag="xself")
                        nc.sync.dma_start(
                            xself[:], xt_shard[b * P:(b + 1) * P, :F0P])
                        acc = bp.tile([P, F0P], dt.float32, tag="acc")
                        nc.vector.tensor_tensor(acc[:], accn[:], xself[:],
                                                op=mybir.AluOpType.add)
                        accs = bp.tile([P, F0P], dt.float32, tag="accs")
                        nc.vector.tensor_scalar_mul(accs[:], acc[:], dv)
                        tp1 = pp.tile([F0P, P], dt.float32, space="PSUM", tag="tp1")
                        nc.tensor.transpose(tp1[:], accs[:], ident[:])
                        tT = bp.tile([F0P, P], dt.float32, tag="tT")
                        nc.scalar.copy(tT[:], tp1[:])
                        h1p = pp.tile([P, F1], dt.float32, space="PSUM", tag="h1p")
                        nc.tensor.matmul(h1p[:], lhsT=tT[:], rhs=w1_s[:],
                                         start=True, stop=True)
                        h1b = bp.tile([P, F1], dt.float32, tag="h1b")
                        nc.vector.tensor_tensor(h1b[:], h1p[:], b1_bc[:],
                                                op=mybir.AluOpType.add)
                        h1t = bp.tile([P, F1], dt.float32, tag="h1t")
                        nc.scalar.activation(h1t[:], h1b[:],
                                             mybir.ActivationFunctionType.Tanh)
                        h1s = bp.tile([P, F1], dt.float32, tag="h1s")
                        nc.vector.tensor_scalar_mul(h1s[:], h1t[:], dv)
                        tp2 = pp.tile([F1, P], dt.float32, space="PSUM", tag="tp2")
                        nc.tensor.transpose(tp2[:], h1s[:], ident[:])
                        t2 = bp.tile([F1, P], dt.float32, tag="t2")
                        nc.scalar.copy(t2[:], tp2[:])
                        gp_ = pp.tile([P, F2], dt.float32, space="PSUM", tag="gp")
                        nc.tensor.matmul(gp_[:], lhsT=t2[:], rhs=w2_s[:],
                                         start=True, stop=True)
                        nc.vector.tensor_copy(gs_w[:, b, :F2], gp_[:])
                # write g~ shard in chunks
                for j0 in range(0, NB, CH):
                    nch = min(CH, NB - j0)
                    dstv = gt_shard[j0 * P:(j0 + nch) * P, :].rearrange(
                        "(j p) f -> p j f", p=P)
                    nc.sync.dma_start(dstv, gs_w[:, j0:j0 + nch, :])
            all_gather(gt_shard[:], gt_full[:])

            # ---------------- phase 4: layer 2 + head + pooling -------------
            with tc.tile_pool(name="g2", bufs=2) as gp2, \
                 tc.tile_pool(name="ix2", bufs=2) as ixp2, \
                 tc.tile_pool(name="blk2", bufs=3) as bp2, \
                 tc.tile_pool(name="ps2", bufs=2, space="PSUM") as pp2:
                for sd in pl.sb_descs:
                    csb = sd["csb"]
                    ic_lo = sd["segs"][0][3]
                    ic_hi = sd["segs"][-1][3] + sd["segs"][-1][2] * 8
                    idx_t = ixp2.tile([P, CSB_MAX * 8], dt.int16, tag="idx2")
                    nc.sync.dma_start(idx_t[:, :ic_hi - ic_lo],
                                      idx_all[:, ic_lo:ic_hi])
                    G2 = gp2.tile([P, CSB_MAX, GW], dt.float32, tag="G2")
                    for (r, gc0, ncols, ic0) in sd["segs"]:
                        for k in range(0, ncols, 8):
                            nc_ = min(8, ncols - k)
                            nidx = nc_ * P
                            nc.gpsimd.dma_gather(
                                G2[:, gc0 + k:gc0 + k + nc_, :],
                                gt_full[r * RSZ:(r + 1) * RSZ, :],
                                idx_t[:, ic0 - ic_lo + k * 8:
                                      ic0 - ic_lo + k * 8 + nc_ * 8],
                                nidx, nidx, GW)
                    for b in sd["blks"]:
                        dv = dinv_all[:, b:b + 1]
                        st2 = bp2.tile([P, NR, F2], dt.float32, tag="st2")
                        nc.vector.memset(st2[:], 0.0)
                        for r in range(NR):
                            Tbr = int(pl.T4[b][r])
                            if Tbr == 0:
                                continue
                            t0 = sd["col_in_sb"][(b, r)]
                            nc.vector.tensor_reduce(
                                st2[:, r, :],
                                G2[:, t0:t0 + Tbr, :F2].transpose([0, 2, 1]),
                                axis=mybir.AxisListType.X, op=mybir.AluOpType.add)
                        a2n = bp2.tile([P, F2], dt.float32, tag="a2n")
                        nc.vector.tensor_reduce(
                            a2n[:], st2[:].transpose([0, 2, 1]),
                            axis=mybir.AxisListType.X, op=mybir.AluOpType.add)
                        gself = bp2.tile([P, F2], dt.float32, tag="gself")
                        nc.sync.dma_start(
                            gself[:], gt_shard[b * P:(b + 1) * P, :F2])
                        acc2 = bp2.tile([P, F2], dt.float32, tag="acc2")
                        nc.vector.tensor_tensor(acc2[:], a2n[:], gself[:],
                                                op=mybir.AluOpType.add)
                        a2s = bp2.tile([P, F2], dt.float32, tag="a2s")
                        nc.vector.tensor_scalar_mul(a2s[:], acc2[:], dv)
                        a2b = bp2.tile([P, F2], dt.float32, tag="a2b")
                        nc.vector.tensor_tensor(a2b[:], a2s[:], b2_bc[:],
                                                op=mybir.AluOpType.add)
                        h2 = bp2.tile([P, F2], dt.float32, tag="h2")
                        nc.scalar.activation(h2[:], a2b[:],
                                             mybir.ActivationFunctionType.Tanh)
                        tp3 = pp2.tile([F2, P], dt.float32, space="PSUM", tag="tp3")
                        nc.tensor.transpose(tp3[:], h2[:], ident[:])
                        hT = bp2.tile([F2, P], dt.float32, tag="hT")
                        nc.scalar.copy(hT[:], tp3[:])
                        zp = pp2.tile([P, 1], dt.float32, space="PSUM", tag="zp")
                        nc.tensor.matmul(zp[:], lhsT=hT[:], rhs=wh_s[:],
                                         start=True, stop=True)
                        zs = bp2.tile([P, 1], dt.float32, tag="zs")
                        nc.vector.tensor_copy(zs[:], zp[:])
                        S = bp2.tile([P, GPC], dt.float32, tag="S")
                        nc.vector.tensor_scalar(
                            S[:], iota_f[:], batchl_all[:, b:b + 1], None,
                            op0=mybir.AluOpType.is_equal)
                        ppsum = pp2.tile([GPC, 1], dt.float32, space="PSUM", tag="pp")
                        nc.tensor.matmul(ppsum[:], lhsT=S[:], rhs=zs[:],
                                         start=True, stop=True)
                        nc.vector.tensor_tensor(util_acc[:], util_acc[:], ppsum[:],
                                                op=mybir.AluOpType.add)

            # ---------------- phase 5: pooling finalize + pair diffs --------
            with tc.tile_pool(name="p5", bufs=1) as p5:
                util0 = p5.tile([GPC, 1], dt.float32)
                nc.vector.tensor_scalar(
                    util0[:], util_acc[:], invcnt_s[:], bh_bc[:],
                    op0=mybir.AluOpType.mult, op1=mybir.AluOpType.add)
                util = p5.tile([GPC, 1], dt.float32)
                nc.vector.tensor_scalar_mul(util[:], util0[:], gmask_s[:])
                nc.sync.dma_start(xu_out[:], util[:])
                nc.sync.dma_start(util_sh[:], util[:])
                all_gather(util_sh[:], util_full[:])
                pa_t = p5.tile([pl.p_use, pl.tp], dt.int32)
                nc.sync.dma_start(pa_t[:], pair_a[:])
                pb_t = p5.tile([pl.p_use, pl.tp], dt.int32)
                nc.sync.dma_start(pb_t[:], pair_b[:])
                ua = p5.tile([pl.p_use, pl.tp, 1], dt.float32)
                ub = p5.tile([pl.p_use, pl.tp, 1], dt.float32)
                for t in range(pl.tp):
                    nc.gpsimd.indirect_dma_start(
                        out=ua[:, t, :], out_offset=None, in_=util_full[:],
                        in_offset=bass.IndirectOffsetOnAxis(ap=pa_t[:, t:t + 1], axis=0))
                    nc.gpsimd.indirect_dma_start(
                        out=ub[:, t, :], out_offset=None, in_=util_full[:],
                        in_offset=bass.IndirectOffsetOnAxis(ap=pb_t[:, t:t + 1], axis=0))
                d = p5.tile([pl.p_use, pl.tp], dt.float32)
                nc.vector.tensor_tensor(d[:], ub[:].squeeze(2), ua[:].squeeze(2),
                                        op=mybir.AluOpType.subtract)
                nc.sync.dma_start(
                    pairs_out[:].rearrange("(p t) o -> p (t o)", p=pl.p_use), d[:])

    nc.compile()
    return nc


# ---------------------------------------------------------------------------
# Entry points
# ---------------------------------------------------------------------------

def prepare(inputs, n_cores=N_CORES):
    """Build plan, program and per-core input maps from full inputs."""
    x = np.asarray(inputs["x"], dtype=np.float32)
    edge_index = np.asarray(inputs["edge_index"])
    batch = np.asarray(inputs["batch"])
    idx_a = np.asarray(inputs["idx_a"])
    idx_b = np.asarray(inputs["idx_b"])
    n_nodes = x.shape[0]
    n_graphs = int(max(batch.max(), idx_a.max(), idx_b.max())) + 1
    n_graphs = ((n_graphs + n_cores - 1) // n_cores) * n_cores
    n_pairs = idx_a.shape[0]

    pl = make_plan(edge_index, batch, n_nodes, n_graphs, n_cores, n_pairs,
                   idx_a, idx_b)

    W1 = np.asarray(inputs["W1"], dtype=np.float32)
    w1p = np.zeros((F0P, F1), dtype=np.float32)
    w1p[:W1.shape[0], :] = W1
    b1 = np.asarray(inputs["b1"], dtype=np.float32)[None, :]
    W2 = np.asarray(inputs["W2"], dtype=np.float32)
    b2 = np.asarray(inputs["b2"], dtype=np.float32)[None, :]
    Wf1T = np.ascontiguousarray(np.asarray(inputs["Wf1"], dtype=np.float32).T)
    Wf2 = np.asarray(inputs["Wf2"], dtype=np.float32).reshape(F2, 1)
    bf1 = np.asarray(inputs["bf1"], dtype=np.float32).reshape(F2, 1)
    bf2 = np.asarray(inputs["bf2"], dtype=np.float32).reshape(1, 1)

    in_maps = []
    for c in range(n_cores):
        xs = np.zeros((pl.NSH, F0P), dtype=np.float32)
        o = pl.order_c[c]
        xs[:len(o), :x.shape[1]] = x[o]
        in_maps.append({
            "x_shard": xs,
            "idx_all": pl.idx_all[c],
            "dinv_pb": pl.dinv_pb[c],
            "batchl_pb": pl.batchl_pb[c],
            "invcnt": pl.invcnt[c],
            "gmask": pl.gmask[c],
            "pair_a": pl.pair_a[c],
            "pair_b": pl.pair_b[c],
            "w1p": w1p, "b1d": b1, "w2d": W2, "b2d": b2,
            "wf1t": Wf1T, "wf2d": Wf2, "bf1d": bf1, "bf2d": bf2,
        })
    nc = build_program(pl)
    return nc, pl, in_maps


def assemble(pl, results):
    n_cores = pl.n_cores
    out = np.concatenate(
        [results[c]["pairs_out"].reshape(-1)[:pl.ppc] for c in range(n_cores)])
    xu = np.concatenate([results[c]["xu_out"] for c in range(n_cores)], axis=0)
    return out.astype(np.float32), xu.astype(np.float32)


def kernel(**inputs):
    nc, pl, in_maps = prepare(inputs, N_CORES)
    res = bass_utils.run_bass_kernel_spmd(nc, in_maps, core_ids=list(range(N_CORES)))
    return assemble(pl, res.results)


# revision 29
# speedup vs baseline: 1.4473x; 1.4473x over previous
"""Trainium2 Bass kernel for RankGNN (2-layer GCN + head + mean-pool + pair diff).

Strategy (edge-parallel, graph-partitioned, per sharding hint):
 - Nodes are sharded across 8 cores at graph boundaries (batch is sorted), so
   pooling is fully core-local. Within a shard, nodes are permuted by degree
   (descending) so that fixed-height "transversal" edge tiles pack densely.
 - Message passing out[i] = dinv_i * sum_{j->i, j=i} xt_j with xt = dinv * x
   is computed by per-dst-block gathers: for a block of 128 dst nodes, slot p
   of every tile holds one in-edge of node p (or a zero row), so aggregation
   is a plain strided sum (VectorE reduce) -- no scatter needed.
 - Gathers use GPSIMD indirect DMA (per-edge row gather from DRAM), with the
   (A x) W reassociation so layer 1 gathers 16-float rows and layer 2 gathers
   32-float rows of g~ = (dinv*h1) @ W2.
 - Cross-core: AllGather of the xt / g~ node shards between layers.
 - Head MLP is folded: z = h2 @ (Wf1@Wf2) + (bf1@Wf2+bf2) (computed on device),
   pooling via one-hot matmul per block, pair diffs via tiny indirect gathers.
"""
import sys
import os

sys.path.insert(0, "/opt/trn_rl_repo")

import numpy as np

import concourse.bass as bass
import concourse.bacc as bacc
import concourse.tile as tile
import concourse.mybir as mybir
from concourse import bass_utils
from concourse.masks import make_identity

P = 128
F0P = 16          # padded input feature dim (9 -> 16)
F1 = 64
F2 = 32

N_CORES = 8
N_NODES = 100000
N_GRAPHS = 1024
N_PAIRS = 4096

CSB_MAX = 192     # max gather columns per superblock
CH = 16           # phase-1/phase-2 dma batching (blocks per chunk)


# ---------------------------------------------------------------------------
# Host-side preprocessing: sharding, degree-sorted blocking, transversal tiles
# ---------------------------------------------------------------------------

class Plan:
    pass


def make_plan(edge_index, batch, n_nodes, n_graphs, n_cores, n_pairs,
              idx_a, idx_b, csb_max=CSB_MAX):
    src = np.asarray(edge_index[0], dtype=np.int64)
    dst = np.asarray(edge_index[1], dtype=np.int64)
    batch = np.asarray(batch, dtype=np.int64)
    E = src.shape[0]

    pl = Plan()
    pl.n_cores = n_cores
    pl.gpc = n_graphs // n_cores          # graphs per core
    pl.ppc = n_pairs // n_cores           # pairs per core
    if pl.ppc >= P:
        assert pl.ppc % P == 0
        pl.p_use, pl.tp = P, pl.ppc // P
    else:
        pl.p_use, pl.tp = pl.ppc, 1

    # node ranges per core (graph-aligned)
    gstart = np.searchsorted(batch, np.arange(0, n_graphs + 1, pl.gpc))
    nodes_c = np.diff(gstart)
    NSH = int((int(nodes_c.max()) + 1 + P - 1) // P) * P
    NB = NSH // P
    pl.NSH, pl.NB = NSH, NB

    deg = np.bincount(dst, minlength=n_nodes).astype(np.int64) + 1  # + self loop
    dinv = (1.0 / np.sqrt(deg)).astype(np.float32)

    # in-edge CSR (by dst)
    eorder = np.argsort(dst, kind="stable")
    srcs_sorted = src[eorder]
    estart = np.zeros(n_nodes + 1, dtype=np.int64)
    np.cumsum(np.bincount(dst, minlength=n_nodes), out=estart[1:])
    indeg = deg - 1

    # per-core degree-desc permutation
    order_c = []
    gpos = np.empty(n_nodes, dtype=np.int64)
    for c in range(n_cores):
        ids = np.arange(gstart[c], gstart[c + 1])
        o = ids[np.argsort(-deg[ids], kind="stable")]
        order_c.append(o)
        gpos[o] = np.arange(len(o)) + c * NSH
    zrow = int(len(order_c[0]))  # first pad row of core 0 (global position)

    # unified tiles-per-block T_b (max over cores; deg sorted desc -> first row max)
    T = np.ones(NB, dtype=np.int64)
    for c in range(n_cores):
        dsort = deg[order_c[c]]
        for b in range(NB):
            lo = b * P
            if lo < len(dsort):
                T[b] = max(T[b], int(dsort[lo]))
    pl.T = T
    pl.C_total = int(T.sum())
    col0 = np.zeros(NB, dtype=np.int64)
    np.cumsum(T[:-1], out=col0[1:])
    pl.col0 = col0

    # superblock grouping
    sbs = []  # (cstart, [block ids])
    cur, cur_cols = [], 0
    for b in range(NB):
        if cur and cur_cols + T[b] > csb_max:
            sbs.append((int(col0[cur[0]]), list(cur)))
            cur, cur_cols = [], 0
        cur.append(b)
        cur_cols += int(T[b])
    if cur:
        sbs.append((int(col0[cur[0]]), list(cur)))
    pl.sbs = sbs

    # per-core data arrays
    pl.idx_all = []
    pl.x_shard = []
    pl.dinv_pb = []
    pl.batchl_pb = []
    pl.invcnt = []
    pl.gmask = []
    pl.pair_a = []
    pl.pair_b = []
    counts = np.bincount(batch, minlength=n_graphs).astype(np.float32)
    for c in range(n_cores):
        o = order_c[c]
        n_c = len(o)
        idx_mat = np.full((P, pl.C_total), zrow, dtype=np.int32)
        dinv_mat = np.zeros((P, NB), dtype=np.float32)
        batchl_mat = np.full((P, NB), -1.0, dtype=np.float32)
        for b in range(NB):
            rows = o[b * P:(b + 1) * P]
            np_rows = len(rows)
            if np_rows == 0:
                continue
            Tb = int(T[b])
            c0 = int(col0[b])
            # col 0: self loop
            idx_mat[:np_rows, c0] = gpos[rows]
            if Tb > 1:
                offs = estart[rows][:, None] + np.arange(Tb - 1)[None, :]
                valid = np.arange(Tb - 1)[None, :] < indeg[rows][:, None]
                g = gpos[srcs_sorted[np.minimum(offs, E - 1)]]
                idx_mat[:np_rows, c0 + 1:c0 + Tb] = np.where(valid, g, zrow)
            dinv_mat[:np_rows, b] = dinv[rows]
            batchl_mat[:np_rows, b] = (batch[rows] - c * pl.gpc).astype(np.float32)
        pl.idx_all.append(idx_mat)
        xs = np.zeros((NSH, F0P), dtype=np.float32)
        pl.x_shard.append(xs)   # feature values filled in kernel() (needs x)
        pl.x_rows = getattr(pl, "x_rows", [])
        pl.dinv_pb.append(dinv_mat)
        pl.batchl_pb.append(batchl_mat)
        cc = counts[c * pl.gpc:(c + 1) * pl.gpc]
        iv = np.where(cc > 0, 1.0 / np.maximum(cc, 1.0), 0.0).astype(np.float32)
        pl.invcnt.append(iv[:, None])
        pl.gmask.append((cc > 0).astype(np.float32)[:, None])
        pa = np.zeros((pl.p_use, pl.tp), dtype=np.int32)
        pb = np.zeros((pl.p_use, pl.tp), dtype=np.int32)
        loc = np.asarray(idx_a[c * pl.ppc:(c + 1) * pl.ppc], dtype=np.int32)
        lob = np.asarray(idx_b[c * pl.ppc:(c + 1) * pl.ppc], dtype=np.int32)
        pa[:, :] = loc.reshape(pl.p_use, pl.tp)
        pb[:, :] = lob.reshape(pl.p_use, pl.tp)
        pl.pair_a.append(pa)
        pl.pair_b.append(pb)
    pl.order_c = order_c
    pl.n_graphs = n_graphs
    return pl


# ---------------------------------------------------------------------------
# Bass program builder (one SPMD program; per-core differences are data-only)
# ---------------------------------------------------------------------------

def build_program(pl, est=False):
    dt = mybir.dt
    NC = pl.n_cores
    NSH, NB = pl.NSH, pl.NB
    GPC = pl.gpc

    nc = bacc.Bacc("TRN2", target_bir_lowering=False, debug=False,
                   num_devices=1 if est else NC)

    def all_gather(in_ap, out_ap):
        if est:
            # cost-estimate single-core build: stand-in copy (collective time
            # is accounted separately)
            nc.sync.dma_start(out_ap[0:in_ap.shape[0]], in_ap)
        else:
            nc.gpsimd.collective_compute(
                "AllGather", mybir.AluOpType.bypass,
                replica_groups=[list(range(NC))],
                ins=[in_ap.opt()], outs=[out_ap.opt()])

    # external inputs
    x_sh = nc.dram_tensor("x_shard", [NSH, F0P], dt.float32, kind="ExternalInput")
    idx_all = nc.dram_tensor("idx_all", [P, pl.C_total], dt.int32, kind="ExternalInput")
    dinv_pb = nc.dram_tensor("dinv_pb", [P, NB], dt.float32, kind="ExternalInput")
    batchl_pb = nc.dram_tensor("batchl_pb", [P, NB], dt.float32, kind="ExternalInput")
    invcnt = nc.dram_tensor("invcnt", [GPC, 1], dt.float32, kind="ExternalInput")
    gmask = nc.dram_tensor("gmask", [GPC, 1], dt.float32, kind="ExternalInput")
    pair_a = nc.dram_tensor("pair_a", [pl.p_use, pl.tp], dt.int32, kind="ExternalInput")
    pair_b = nc.dram_tensor("pair_b", [pl.p_use, pl.tp], dt.int32, kind="ExternalInput")
    w1p = nc.dram_tensor("w1p", [F0P, F1], dt.float32, kind="ExternalInput")
    b1d = nc.dram_tensor("b1d", [1, F1], dt.float32, kind="ExternalInput")
    w2d = nc.dram_tensor("w2d", [F1, F2], dt.float32, kind="ExternalInput")
    b2d = nc.dram_tensor("b2d", [1, F2], dt.float32, kind="ExternalInput")
    wf1t = nc.dram_tensor("wf1t", [F2, F2], dt.float32, kind="ExternalInput")
    wf2d = nc.dram_tensor("wf2d", [F2, 1], dt.float32, kind="ExternalInput")
    bf1d = nc.dram_tensor("bf1d", [F2, 1], dt.float32, kind="ExternalInput")
    bf2d = nc.dram_tensor("bf2d", [1, 1], dt.float32, kind="ExternalInput")

    # external outputs
    xu_out = nc.dram_tensor("xu_out", [GPC, 1], dt.float32, kind="ExternalOutput")
    pairs_out = nc.dram_tensor("pairs_out", [pl.ppc, 1], dt.float32, kind="ExternalOutput")

    # internal dram
    xt_shard = nc.dram_tensor("xt_shard", [NSH, F0P], dt.float32, kind="Internal")
    xt_full = nc.dram_tensor("xt_full", [NC * NSH, F0P], dt.float32, kind="Internal",
                             addr_space="Shared")
    gt_shard = nc.dram_tensor("gt_shard", [NSH, F2], dt.float32, kind="Internal")
    gt_full = nc.dram_tensor("gt_full", [NC * NSH, F2], dt.float32, kind="Internal",
                             addr_space="Shared")
    util_sh = nc.dram_tensor("util_sh", [GPC, 1], dt.float32, kind="Internal")
    util_full = nc.dram_tensor("util_full", [NC * GPC, 1], dt.float32, kind="Internal",
                               addr_space="Shared")
    bh_dram = nc.dram_tensor("bh_dram", [1, 1], dt.float32, kind="Internal")

    groups = [list(range(NC))]

    with tile.TileContext(nc) as tc:
        with tc.tile_pool(name="const", bufs=1) as cp:
            # constants in SBUF
            ident = cp.tile([P, P], dt.float32)
            make_identity(nc, ident[:])
            w1_s = cp.tile([F0P, F1], dt.float32)
            nc.sync.dma_start(w1_s[:], w1p[:])
            w2_s = cp.tile([F1, F2], dt.float32)
            nc.sync.dma_start(w2_s[:], w2d[:])
            wf1t_s = cp.tile([F2, F2], dt.float32)
            nc.sync.dma_start(wf1t_s[:], wf1t[:])
            wf2_s = cp.tile([F2, 1], dt.float32)
            nc.sync.dma_start(wf2_s[:], wf2d[:])
            bf1_s = cp.tile([F2, 1], dt.float32)
            nc.sync.dma_start(bf1_s[:], bf1d[:])
            bf2_s = cp.tile([1, 1], dt.float32)
            nc.sync.dma_start(bf2_s[:], bf2d[:])
            b1_bc = cp.tile([P, F1], dt.float32)
            nc.sync.dma_start(b1_bc[:], b1d[:].partition_broadcast(P))
            b2_bc = cp.tile([P, F2], dt.float32)
            nc.sync.dma_start(b2_bc[:], b2d[:].partition_broadcast(P))
            dinv_all = cp.tile([P, NB], dt.float32)
            nc.sync.dma_start(dinv_all[:], dinv_pb[:])
            batchl_all = cp.tile([P, NB], dt.float32)
            nc.sync.dma_start(batchl_all[:], batchl_pb[:])
            invcnt_s = cp.tile([GPC, 1], dt.float32)
            nc.sync.dma_start(invcnt_s[:], invcnt[:])
            gmask_s = cp.tile([GPC, 1], dt.float32)
            nc.sync.dma_start(gmask_s[:], gmask[:])
            iota_i = cp.tile([P, GPC], dt.int32)
            nc.gpsimd.iota(iota_i[:], pattern=[[1, GPC]], base=0, channel_multiplier=0)
            iota_f = cp.tile([P, GPC], dt.float32)
            nc.vector.tensor_copy(iota_f[:], iota_i[:])

            # folded head: w_head = Wf1 @ Wf2, b_head = bf1.Wf2 + bf2
            wh_s = cp.tile([F2, 1], dt.float32)
            bh_s = cp.tile([1, 1], dt.float32)
            with tc.tile_pool(name="cpsum", bufs=1, space="PSUM") as cpp:
                wh_p = cpp.tile([F2, 1], dt.float32, space="PSUM")
                nc.tensor.matmul(wh_p[:], lhsT=wf1t_s[:], rhs=wf2_s[:], start=True, stop=True)
                nc.vector.tensor_copy(wh_s[:], wh_p[:])
                bh_p = cpp.tile([1, 1], dt.float32, space="PSUM")
                nc.tensor.matmul(bh_p[:], lhsT=bf1_s[:], rhs=wf2_s[:], start=True, stop=True)
                nc.vector.tensor_tensor(bh_s[:], bh_p[:], bf2_s[:], op=mybir.AluOpType.add)
            nc.sync.dma_start(bh_dram[:], bh_s[:])
            bh_bc = cp.tile([GPC, 1], dt.float32)
            nc.sync.dma_start(bh_bc[:], bh_dram[:].partition_broadcast(GPC))

            util_acc = cp.tile([GPC, 1], dt.float32)
            nc.vector.memset(util_acc[:], 0.0)

            # ---------------- phase 1: xt = dinv * x (shard), AllGather ------
            with tc.tile_pool(name="p1", bufs=2) as p1:
                for j0 in range(0, NB, CH):
                    nch = min(CH, NB - j0)
                    xch = p1.tile([P, CH, F0P], dt.float32, tag="xch")
                    src_view = x_sh[j0 * P:(j0 + nch) * P, :].rearrange(
                        "(j p) f -> p j f", p=P)
                    nc.sync.dma_start(xch[:, :nch, :], src_view)
                    xs = p1.tile([P, CH, F0P], dt.float32, tag="xs")
                    dv = dinv_all[:, j0:j0 + nch].unsqueeze(2).to_broadcast(
                        [P, nch, F0P])
                    nc.vector.tensor_tensor(xs[:, :nch, :], xch[:, :nch, :], dv,
                                            op=mybir.AluOpType.mult)
                    dst_view = xt_shard[j0 * P:(j0 + nch) * P, :].rearrange(
                        "(j p) f -> p j f", p=P)
                    nc.sync.dma_start(dst_view, xs[:, :nch, :])
            all_gather(xt_shard[:], xt_full[:])

            # ---------------- phase 2: layer 1 + g~ ------------------------
            with tc.tile_pool(name="g1", bufs=2) as gp1, \
                 tc.tile_pool(name="ix1", bufs=2) as ixp, \
                 tc.tile_pool(name="blk1", bufs=3) as bp, \
                 tc.tile_pool(name="gs8", bufs=2) as gsp, \
                 tc.tile_pool(name="ps1", bufs=2, space="PSUM") as pp:
                gs_w = gsp.tile([P, NB, F2], dt.float32, tag="gsall")
                for cstart, blks in pl.sbs:
                    csb = int(sum(pl.T[b] for b in blks))
                    idx_t = ixp.tile([P, CSB_MAX], dt.int32, tag="idx")
                    nc.sync.dma_start(idx_t[:, :csb], idx_all[:, cstart:cstart + csb])
                    G = gp1.tile([P, CSB_MAX, F0P], dt.float32, tag="G")
                    for c in range(csb):
                        nc.gpsimd.indirect_dma_start(
                            out=G[:, c, :], out_offset=None,
                            in_=xt_full[:],
                            in_offset=bass.IndirectOffsetOnAxis(
                                ap=idx_t[:, c:c + 1], axis=0),
                        )
                    for b in blks:
                        t0 = int(pl.col0[b]) - cstart
                        Tb = int(pl.T[b])
                        dv = dinv_all[:, b:b + 1]
                        acc = bp.tile([P, F0P], dt.float32, tag="acc")
                        nc.vector.tensor_reduce(
                            acc[:], G[:, t0:t0 + Tb, :].transpose([0, 2, 1]),
                            axis=mybir.AxisListType.X, op=mybir.AluOpType.add)
                        accs = bp.tile([P, F0P], dt.float32, tag="accs")
                        nc.vector.tensor_scalar_mul(accs[:], acc[:], dv)
                        tp1 = pp.tile([F0P, P], dt.float32, space="PSUM", tag="tp1")
                        nc.tensor.transpose(tp1[:], accs[:], ident[:])
                        tT = bp.tile([F0P, P], dt.float32, tag="tT")
                        nc.scalar.copy(tT[:], tp1[:])
                        h1p = pp.tile([P, F1], dt.float32, space="PSUM", tag="h1p")
                        nc.tensor.matmul(h1p[:], lhsT=tT[:], rhs=w1_s[:],
                                         start=True, stop=True)
                        h1b = bp.tile([P, F1], dt.float32, tag="h1b")
                        nc.vector.tensor_tensor(h1b[:], h1p[:], b1_bc[:],
                                                op=mybir.AluOpType.add)
                        h1t = bp.tile([P, F1], dt.float32, tag="h1t")
                        nc.scalar.activation(h1t[:], h1b[:],
                                             mybir.ActivationFunctionType.Tanh)
                        h1s = bp.tile([P, F1], dt.float32, tag="h1s")
                        nc.vector.tensor_scalar_mul(h1s[:], h1t[:], dv)
                        tp2 = pp.tile([F1, P], dt.float32, space="PSUM", tag="tp2")
                        nc.tensor.transpose(tp2[:], h1s[:], ident[:])
                        t2 = bp.tile([F1, P], dt.float32, tag="t2")
                        nc.scalar.copy(t2[:], tp2[:])
                        gp_ = pp.tile([P, F2], dt.float32, space="PSUM", tag="gp")
                        nc.tensor.matmul(gp_[:], lhsT=t2[:], rhs=w2_s[:],
                                         start=True, stop=True)
                        nc.vector.tensor_copy(gs_w[:, b, :], gp_[:])
                # write g~ shard in chunks
                for j0 in range(0, NB, CH):
                    nch = min(CH, NB - j0)
                    dstv = gt_shard[j0 * P:(j0 + nch) * P, :].rearrange(
                        "(j p) f -> p j f", p=P)
                    nc.sync.dma_start(dstv, gs_w[:, j0:j0 + nch, :])
            all_gather(gt_shard[:], gt_full[:])

            # ---------------- phase 4: layer 2 + head + pooling -------------
            with tc.tile_pool(name="g2", bufs=2) as gp2, \
                 tc.tile_pool(name="ix2", bufs=2) as ixp2, \
                 tc.tile_pool(name="blk2", bufs=3) as bp2, \
                 tc.tile_pool(name="ps2", bufs=2, space="PSUM") as pp2:
                for cstart, blks in pl.sbs:
                    csb = int(sum(pl.T[b] for b in blks))
                    idx_t = ixp2.tile([P, CSB_MAX], dt.int32, tag="idx2")
                    nc.sync.dma_start(idx_t[:, :csb], idx_all[:, cstart:cstart + csb])
                    G2 = gp2.tile([P, CSB_MAX, F2], dt.float32, tag="G2")
                    for c in range(csb):
                        nc.gpsimd.indirect_dma_start(
                            out=G2[:, c, :], out_offset=None,
                            in_=gt_full[:],
                            in_offset=bass.IndirectOffsetOnAxis(
                                ap=idx_t[:, c:c + 1], axis=0),
                        )
                    for b in blks:
                        t0 = int(pl.col0[b]) - cstart
                        Tb = int(pl.T[b])
                        dv = dinv_all[:, b:b + 1]
                        acc2 = bp2.tile([P, F2], dt.float32, tag="acc2")
                        nc.vector.tensor_reduce(
                            acc2[:], G2[:, t0:t0 + Tb, :].transpose([0, 2, 1]),
                            axis=mybir.AxisListType.X, op=mybir.AluOpType.add)
                        a2s = bp2.tile([P, F2], dt.float32, tag="a2s")
                        nc.vector.tensor_scalar_mul(a2s[:], acc2[:], dv)
                        a2b = bp2.tile([P, F2], dt.float32, tag="a2b")
                        nc.vector.tensor_tensor(a2b[:], a2s[:], b2_bc[:],
                                                op=mybir.AluOpType.add)
                        h2 = bp2.tile([P, F2], dt.float32, tag="h2")
                        nc.scalar.activation(h2[:], a2b[:],
                                             mybir.ActivationFunctionType.Tanh)
                        tp3 = pp2.tile([F2, P], dt.float32, space="PSUM", tag="tp3")
                        nc.tensor.transpose(tp3[:], h2[:], ident[:])
                        hT = bp2.tile([F2, P], dt.float32, tag="hT")
                        nc.scalar.copy(hT[:], tp3[:])
                        zp = pp2.tile([P, 1], dt.float32, space="PSUM", tag="zp")
                        nc.tensor.matmul(zp[:], lhsT=hT[:], rhs=wh_s[:],
                                         start=True, stop=True)
                        zs = bp2.tile([P, 1], dt.float32, tag="zs")
                        nc.vector.tensor_copy(zs[:], zp[:])
                        S = bp2.tile([P, GPC], dt.float32, tag="S")
                        nc.vector.tensor_scalar(
                            S[:], iota_f[:], batchl_all[:, b:b + 1], None,
                            op0=mybir.AluOpType.is_equal)
                        ppsum = pp2.tile([GPC, 1], dt.float32, space="PSUM", tag="pp")
                        nc.tensor.matmul(ppsum[:], lhsT=S[:], rhs=zs[:],
                                         start=True, stop=True)
                        nc.vector.tensor_tensor(util_acc[:], util_acc[:], ppsum[:],
                                                op=mybir.AluOpType.add)

            # ---------------- phase 5: pooling finalize + pair diffs --------
            with tc.tile_pool(name="p5", bufs=1) as p5:
                util0 = p5.tile([GPC, 1], dt.float32)
                nc.vector.tensor_scalar(
                    util0[:], util_acc[:], invcnt_s[:], bh_bc[:],
                    op0=mybir.AluOpType.mult, op1=mybir.AluOpType.add)
                util = p5.tile([GPC, 1], dt.float32)
                nc.vector.tensor_scalar_mul(util[:], util0[:], gmask_s[:])
                nc.sync.dma_start(xu_out[:], util[:])
                nc.sync.dma_start(util_sh[:], util[:])
                all_gather(util_sh[:], util_full[:])
                pa_t = p5.tile([pl.p_use, pl.tp], dt.int32)
                nc.sync.dma_start(pa_t[:], pair_a[:])
                pb_t = p5.tile([pl.p_use, pl.tp], dt.int32)
                nc.sync.dma_start(pb_t[:], pair_b[:])
                ua = p5.tile([pl.p_use, pl.tp, 1], dt.float32)
                ub = p5.tile([pl.p_use, pl.tp, 1], dt.float32)
                for t in range(pl.tp):
                    nc.gpsimd.indirect_dma_start(
                        out=ua[:, t, :], out_offset=None, in_=util_full[:],
                        in_offset=bass.IndirectOffsetOnAxis(ap=pa_t[:, t:t + 1], axis=0))
                    nc.gpsimd.indirect_dma_start(
                        out=ub[:, t, :], out_offset=None, in_=util_full[:],
                        in_offset=bass.IndirectOffsetOnAxis(ap=pb_t[:, t:t + 1], axis=0))
                d = p5.tile([pl.p_use, pl.tp], dt.float32)
                nc.vector.tensor_tensor(d[:], ub[:].squeeze(2), ua[:].squeeze(2),
                                        op=mybir.AluOpType.subtract)
                nc.sync.dma_start(
                    pairs_out[:].rearrange("(p t) o -> p (t o)", p=pl.p_use), d[:])

    nc.compile()
    return nc


# ---------------------------------------------------------------------------
# Entry points
# ---------------------------------------------------------------------------

def prepare(inputs, n_cores=N_CORES):
    """Build plan, program and per-core input maps from full inputs."""
    x = np.asarray(inputs["x"], dtype=np.float32)
    edge_index = np.asarray(inputs["edge_index"])
    batch = np.asarray(inputs["batch"])
    idx_a = np.asarray(inputs["idx_a"])
    idx_b = np.asarray(inputs["idx_b"])
    n_nodes = x.shape[0]
    n_graphs = int(max(batch.max(), idx_a.max(), idx_b.max())) + 1
    n_graphs = ((n_graphs + n_cores - 1) // n_cores) * n_cores
    n_pairs = idx_a.shape[0]

    pl = make_plan(edge_index, batch, n_nodes, n_graphs, n_cores, n_pairs,
                   idx_a, idx_b)

    W1 = np.asarray(inputs["W1"], dtype=np.float32)
    w1p = np.zeros((F0P, F1), dtype=np.float32)
    w1p[:W1.shape[0], :] = W1
    b1 = np.asarray(inputs["b1"], dtype=np.float32)[None, :]
    W2 = np.asarray(inputs["W2"], dtype=np.float32)
    b2 = np.asarray(inputs["b2"], dtype=np.float32)[None, :]
    Wf1T = np.ascontiguousarray(np.asarray(inputs["Wf1"], dtype=np.float32).T)
    Wf2 = np.asarray(inputs["Wf2"], dtype=np.float32).reshape(F2, 1)
    bf1 = np.asarray(inputs["bf1"], dtype=np.float32).reshape(F2, 1)
    bf2 = np.asarray(inputs["bf2"], dtype=np.float32).reshape(1, 1)

    in_maps = []
    for c in range(n_cores):
        xs = np.zeros((pl.NSH, F0P), dtype=np.float32)
        o = pl.order_c[c]
        xs[:len(o), :x.shape[1]] = x[o]
        in_maps.append({
            "x_shard": xs,
            "idx_all": pl.idx_all[c],
            "dinv_pb": pl.dinv_pb[c],
            "batchl_pb": pl.batchl_pb[c],
            "invcnt": pl.invcnt[c],
            "gmask": pl.gmask[c],
            "pair_a": pl.pair_a[c],
            "pair_b": pl.pair_b[c],
            "w1p": w1p, "b1d": b1, "w2d": W2, "b2d": b2,
            "wf1t": Wf1T, "wf2d": Wf2, "bf1d": bf1, "bf2d": bf2,
        })
    nc = build_program(pl)
    return nc, pl, in_maps


def assemble(pl, results):
    n_cores = pl.n_cores
    out = np.concatenate(
        [results[c]["pairs_out"].reshape(-1)[:pl.ppc] for c in range(n_cores)])
    xu = np.concatenate([results[c]["xu_out"] for c in range(n_cores)], axis=0)
    return out.astype(np.float32), xu.astype(np.float32)


def kernel(**inputs):
    nc, pl, in_maps = prepare(inputs, N_CORES)
    res = bass_utils.run_bass_kernel_spmd(nc, in_maps, core_ids=list(range(N_CORES)))
    return assemble(pl, res.results)


# revision 30
# speedup vs baseline: 1.6983x; 1.1734x over previous
"""Trainium2 Bass kernel for RankGNN (2-layer GCN + head + mean-pool + pair diff).

Strategy (edge-parallel, graph-partitioned, per sharding hint):
 - Nodes are sharded across 8 cores at graph boundaries (batch is sorted), so
   pooling is fully core-local. Within a shard, nodes are permuted by degree
   (descending) so that fixed-height "transversal" edge tiles pack densely.
 - Message passing out[i] = dinv_i * sum_{j->i, j=i} xt_j with xt = dinv * x
   is computed by per-dst-block gathers: for a block of 128 dst nodes, slot p
   of every tile holds one in-edge of node p (or a zero row), so aggregation
   is a plain strided sum (VectorE reduce) -- no scatter needed.
 - Gathers use GPSIMD indirect DMA (per-edge row gather from DRAM), with the
   (A x) W reassociation so layer 1 gathers 16-float rows and layer 2 gathers
   32-float rows of g~ = (dinv*h1) @ W2.
 - Cross-core: AllGather of the xt / g~ node shards between layers.
 - Head MLP is folded: z = h2 @ (Wf1@Wf2) + (bf1@Wf2+bf2) (computed on device),
   pooling via one-hot matmul per block, pair diffs via tiny indirect gathers.
"""
import sys
import os

sys.path.insert(0, "/opt/trn_rl_repo")

import numpy as np

import concourse.bass as bass
import concourse.bacc as bacc
import concourse.tile as tile
import concourse.mybir as mybir
from concourse import bass_utils
from concourse.masks import make_identity

P = 128
F0P = 16          # padded input feature dim (9 -> 16)
F1 = 64
F2 = 32

N_CORES = 8
N_NODES = 100000
N_GRAPHS = 1024
N_PAIRS = 4096

CSB_MAX = 192     # max gather columns per superblock
CH = 16           # phase-1/phase-2 dma batching (blocks per chunk)


# ---------------------------------------------------------------------------
# Host-side preprocessing: sharding, degree-sorted blocking, transversal tiles
# ---------------------------------------------------------------------------

class Plan:
    pass


def make_plan(edge_index, batch, n_nodes, n_graphs, n_cores, n_pairs,
              idx_a, idx_b, csb_max=CSB_MAX):
    src = np.asarray(edge_index[0], dtype=np.int64)
    dst = np.asarray(edge_index[1], dtype=np.int64)
    batch = np.asarray(batch, dtype=np.int64)
    E = src.shape[0]

    pl = Plan()
    pl.n_cores = n_cores
    pl.gpc = n_graphs // n_cores          # graphs per core
    pl.ppc = n_pairs // n_cores           # pairs per core
    if pl.ppc >= P:
        assert pl.ppc % P == 0
        pl.p_use, pl.tp = P, pl.ppc // P
    else:
        pl.p_use, pl.tp = pl.ppc, 1

    # node ranges per core (graph-aligned)
    gstart = np.searchsorted(batch, np.arange(0, n_graphs + 1, pl.gpc))
    nodes_c = np.diff(gstart)
    NSH = int((int(nodes_c.max()) + 1 + P - 1) // P) * P
    NB = NSH // P
    pl.NSH, pl.NB = NSH, NB

    deg = np.bincount(dst, minlength=n_nodes).astype(np.int64) + 1  # + self loop
    dinv = (1.0 / np.sqrt(deg)).astype(np.float32)

    # in-edge CSR (by dst)
    eorder = np.argsort(dst, kind="stable")
    srcs_sorted = src[eorder]
    estart = np.zeros(n_nodes + 1, dtype=np.int64)
    np.cumsum(np.bincount(dst, minlength=n_nodes), out=estart[1:])
    indeg = deg - 1

    # per-core degree-desc permutation
    order_c = []
    gpos = np.empty(n_nodes, dtype=np.int64)
    for c in range(n_cores):
        ids = np.arange(gstart[c], gstart[c + 1])
        o = ids[np.argsort(-deg[ids], kind="stable")]
        order_c.append(o)
        gpos[o] = np.arange(len(o)) + c * NSH
    zrow = int(len(order_c[0]))  # first pad row of core 0 (global position)

    # unified tiles-per-block T_b (max over cores; deg sorted desc -> first row max)
    T = np.ones(NB, dtype=np.int64)
    for c in range(n_cores):
        dsort = deg[order_c[c]]
        for b in range(NB):
            lo = b * P
            if lo < len(dsort):
                T[b] = max(T[b], int(dsort[lo]))
    pl.T = T
    pl.C_total = int(T.sum())
    col0 = np.zeros(NB, dtype=np.int64)
    np.cumsum(T[:-1], out=col0[1:])
    pl.col0 = col0

    # superblock grouping
    sbs = []  # (cstart, [block ids])
    cur, cur_cols = [], 0
    for b in range(NB):
        if cur and cur_cols + T[b] > csb_max:
            sbs.append((int(col0[cur[0]]), list(cur)))
            cur, cur_cols = [], 0
        cur.append(b)
        cur_cols += int(T[b])
    if cur:
        sbs.append((int(col0[cur[0]]), list(cur)))
    pl.sbs = sbs

    # per-core data arrays
    pl.idx_all = []
    pl.x_shard = []
    pl.dinv_pb = []
    pl.batchl_pb = []
    pl.invcnt = []
    pl.gmask = []
    pl.pair_a = []
    pl.pair_b = []
    counts = np.bincount(batch, minlength=n_graphs).astype(np.float32)
    for c in range(n_cores):
        o = order_c[c]
        n_c = len(o)
        idx_mat = np.full((P, pl.C_total), zrow, dtype=np.int32)
        dinv_mat = np.zeros((P, NB), dtype=np.float32)
        batchl_mat = np.full((P, NB), -1.0, dtype=np.float32)
        for b in range(NB):
            rows = o[b * P:(b + 1) * P]
            np_rows = len(rows)
            if np_rows == 0:
                continue
            Tb = int(T[b])
            c0 = int(col0[b])
            # col 0: self loop
            idx_mat[:np_rows, c0] = gpos[rows]
            if Tb > 1:
                offs = estart[rows][:, None] + np.arange(Tb - 1)[None, :]
                valid = np.arange(Tb - 1)[None, :] < indeg[rows][:, None]
                g = gpos[srcs_sorted[np.minimum(offs, E - 1)]]
                idx_mat[:np_rows, c0 + 1:c0 + Tb] = np.where(valid, g, zrow)
            dinv_mat[:np_rows, b] = dinv[rows]
            batchl_mat[:np_rows, b] = (batch[rows] - c * pl.gpc).astype(np.float32)
        pl.idx_all.append(idx_mat)
        xs = np.zeros((NSH, F0P), dtype=np.float32)
        pl.x_shard.append(xs)   # feature values filled in kernel() (needs x)
        pl.x_rows = getattr(pl, "x_rows", [])
        pl.dinv_pb.append(dinv_mat)
        pl.batchl_pb.append(batchl_mat)
        cc = counts[c * pl.gpc:(c + 1) * pl.gpc]
        iv = np.where(cc > 0, 1.0 / np.maximum(cc, 1.0), 0.0).astype(np.float32)
        pl.invcnt.append(iv[:, None])
        pl.gmask.append((cc > 0).astype(np.float32)[:, None])
        pa = np.zeros((pl.p_use, pl.tp), dtype=np.int32)
        pb = np.zeros((pl.p_use, pl.tp), dtype=np.int32)
        loc = np.asarray(idx_a[c * pl.ppc:(c + 1) * pl.ppc], dtype=np.int32)
        lob = np.asarray(idx_b[c * pl.ppc:(c + 1) * pl.ppc], dtype=np.int32)
        pa[:, :] = loc.reshape(pl.p_use, pl.tp)
        pb[:, :] = lob.reshape(pl.p_use, pl.tp)
        pl.pair_a.append(pa)
        pl.pair_b.append(pb)
    pl.order_c = order_c
    pl.n_graphs = n_graphs
    return pl


# ---------------------------------------------------------------------------
# Bass program builder (one SPMD program; per-core differences are data-only)
# ---------------------------------------------------------------------------

def build_program(pl, est=False):
    dt = mybir.dt
    NC = pl.n_cores
    NSH, NB = pl.NSH, pl.NB
    GPC = pl.gpc

    nc = bacc.Bacc("TRN2", target_bir_lowering=False, debug=False,
                   num_devices=1 if est else NC)

    def all_gather(in_ap, out_ap):
        if est:
            # cost-estimate single-core build: stand-in copy (collective time
            # is accounted separately)
            nc.sync.dma_start(out_ap[0:in_ap.shape[0]], in_ap)
        else:
            nc.gpsimd.collective_compute(
                "AllGather", mybir.AluOpType.bypass,
                replica_groups=[list(range(NC))],
                ins=[in_ap.opt()], outs=[out_ap.opt()])

    # external inputs
    x_sh = nc.dram_tensor("x_shard", [NSH, F0P], dt.float32, kind="ExternalInput")
    idx_all = nc.dram_tensor("idx_all", [P, pl.C_total], dt.int32, kind="ExternalInput")
    dinv_pb = nc.dram_tensor("dinv_pb", [P, NB], dt.float32, kind="ExternalInput")
    batchl_pb = nc.dram_tensor("batchl_pb", [P, NB], dt.float32, kind="ExternalInput")
    invcnt = nc.dram_tensor("invcnt", [GPC, 1], dt.float32, kind="ExternalInput")
    gmask = nc.dram_tensor("gmask", [GPC, 1], dt.float32, kind="ExternalInput")
    pair_a = nc.dram_tensor("pair_a", [pl.p_use, pl.tp], dt.int32, kind="ExternalInput")
    pair_b = nc.dram_tensor("pair_b", [pl.p_use, pl.tp], dt.int32, kind="ExternalInput")
    w1p = nc.dram_tensor("w1p", [F0P, F1], dt.float32, kind="ExternalInput")
    b1d = nc.dram_tensor("b1d", [1, F1], dt.float32, kind="ExternalInput")
    w2d = nc.dram_tensor("w2d", [F1, F2], dt.float32, kind="ExternalInput")
    b2d = nc.dram_tensor("b2d", [1, F2], dt.float32, kind="ExternalInput")
    wf1t = nc.dram_tensor("wf1t", [F2, F2], dt.float32, kind="ExternalInput")
    wf2d = nc.dram_tensor("wf2d", [F2, 1], dt.float32, kind="ExternalInput")
    bf1d = nc.dram_tensor("bf1d", [F2, 1], dt.float32, kind="ExternalInput")
    bf2d = nc.dram_tensor("bf2d", [1, 1], dt.float32, kind="ExternalInput")

    # external outputs
    xu_out = nc.dram_tensor("xu_out", [GPC, 1], dt.float32, kind="ExternalOutput")
    pairs_out = nc.dram_tensor("pairs_out", [pl.ppc, 1], dt.float32, kind="ExternalOutput")

    # internal dram
    xt_shard = nc.dram_tensor("xt_shard", [NSH, F0P], dt.float32, kind="Internal")
    xt_full = nc.dram_tensor("xt_full", [NC * NSH, F0P], dt.float32, kind="Internal",
                             addr_space="Shared")
    gt_shard = nc.dram_tensor("gt_shard", [NSH, F2], dt.float32, kind="Internal")
    gt_full = nc.dram_tensor("gt_full", [NC * NSH, F2], dt.float32, kind="Internal",
                             addr_space="Shared")
    util_sh = nc.dram_tensor("util_sh", [GPC, 1], dt.float32, kind="Internal")
    util_full = nc.dram_tensor("util_full", [NC * GPC, 1], dt.float32, kind="Internal",
                               addr_space="Shared")
    bh_dram = nc.dram_tensor("bh_dram", [1, 1], dt.float32, kind="Internal")

    groups = [list(range(NC))]

    with tile.TileContext(nc) as tc:
        with tc.tile_pool(name="const", bufs=1) as cp:
            # constants in SBUF
            ident = cp.tile([P, P], dt.float32)
            make_identity(nc, ident[:])
            w1_s = cp.tile([F0P, F1], dt.float32)
            nc.sync.dma_start(w1_s[:], w1p[:])
            w2_s = cp.tile([F1, F2], dt.float32)
            nc.sync.dma_start(w2_s[:], w2d[:])
            wf1t_s = cp.tile([F2, F2], dt.float32)
            nc.sync.dma_start(wf1t_s[:], wf1t[:])
            wf2_s = cp.tile([F2, 1], dt.float32)
            nc.sync.dma_start(wf2_s[:], wf2d[:])
            bf1_s = cp.tile([F2, 1], dt.float32)
            nc.sync.dma_start(bf1_s[:], bf1d[:])
            bf2_s = cp.tile([1, 1], dt.float32)
            nc.sync.dma_start(bf2_s[:], bf2d[:])
            b1_bc = cp.tile([P, F1], dt.float32)
            nc.sync.dma_start(b1_bc[:], b1d[:].partition_broadcast(P))
            b2_bc = cp.tile([P, F2], dt.float32)
            nc.sync.dma_start(b2_bc[:], b2d[:].partition_broadcast(P))
            dinv_all = cp.tile([P, NB], dt.float32)
            nc.sync.dma_start(dinv_all[:], dinv_pb[:])
            batchl_all = cp.tile([P, NB], dt.float32)
            nc.sync.dma_start(batchl_all[:], batchl_pb[:])
            invcnt_s = cp.tile([GPC, 1], dt.float32)
            nc.sync.dma_start(invcnt_s[:], invcnt[:])
            gmask_s = cp.tile([GPC, 1], dt.float32)
            nc.sync.dma_start(gmask_s[:], gmask[:])
            iota_i = cp.tile([P, GPC], dt.int32)
            nc.gpsimd.iota(iota_i[:], pattern=[[1, GPC]], base=0, channel_multiplier=0)
            iota_f = cp.tile([P, GPC], dt.float32)
            nc.vector.tensor_copy(iota_f[:], iota_i[:])

            # folded head: w_head = Wf1 @ Wf2, b_head = bf1.Wf2 + bf2
            wh_s = cp.tile([F2, 1], dt.float32)
            bh_s = cp.tile([1, 1], dt.float32)
            with tc.tile_pool(name="cpsum", bufs=1, space="PSUM") as cpp:
                wh_p = cpp.tile([F2, 1], dt.float32, space="PSUM")
                nc.tensor.matmul(wh_p[:], lhsT=wf1t_s[:], rhs=wf2_s[:], start=True, stop=True)
                nc.vector.tensor_copy(wh_s[:], wh_p[:])
                bh_p = cpp.tile([1, 1], dt.float32, space="PSUM")
                nc.tensor.matmul(bh_p[:], lhsT=bf1_s[:], rhs=wf2_s[:], start=True, stop=True)
                nc.vector.tensor_tensor(bh_s[:], bh_p[:], bf2_s[:], op=mybir.AluOpType.add)
            nc.sync.dma_start(bh_dram[:], bh_s[:])
            bh_bc = cp.tile([GPC, 1], dt.float32)
            nc.sync.dma_start(bh_bc[:], bh_dram[:].partition_broadcast(GPC))

            util_acc = cp.tile([GPC, 1], dt.float32)
            nc.vector.memset(util_acc[:], 0.0)

            # ---------------- phase 1: xt = dinv * x (shard), AllGather ------
            with tc.tile_pool(name="p1", bufs=2) as p1:
                for j0 in range(0, NB, CH):
                    nch = min(CH, NB - j0)
                    xch = p1.tile([P, CH, F0P], dt.float32, tag="xch")
                    src_view = x_sh[j0 * P:(j0 + nch) * P, :].rearrange(
                        "(j p) f -> p j f", p=P)
                    nc.sync.dma_start(xch[:, :nch, :], src_view)
                    xs = p1.tile([P, CH, F0P], dt.float32, tag="xs")
                    dv = dinv_all[:, j0:j0 + nch].unsqueeze(2).to_broadcast(
                        [P, nch, F0P])
                    nc.vector.tensor_tensor(xs[:, :nch, :], xch[:, :nch, :], dv,
                                            op=mybir.AluOpType.mult)
                    dst_view = xt_shard[j0 * P:(j0 + nch) * P, :].rearrange(
                        "(j p) f -> p j f", p=P)
                    nc.sync.dma_start(dst_view, xs[:, :nch, :])
            all_gather(xt_shard[:], xt_full[:])

            # ---------------- phase 2: layer 1 + g~ ------------------------
            with tc.tile_pool(name="g1", bufs=3) as gp1, \
                 tc.tile_pool(name="ix1", bufs=3) as ixp, \
                 tc.tile_pool(name="blk1", bufs=4) as bp, \
                 tc.tile_pool(name="gs8", bufs=2) as gsp, \
                 tc.tile_pool(name="ps1", bufs=2, space="PSUM") as pp:
                gs_w = gsp.tile([P, NB, F2], dt.float32, tag="gsall")
                for cstart, blks in pl.sbs:
                    csb = int(sum(pl.T[b] for b in blks))
                    idx_t = ixp.tile([P, CSB_MAX], dt.int32, tag="idx")
                    nc.sync.dma_start(idx_t[:, :csb], idx_all[:, cstart:cstart + csb])
                    G = gp1.tile([P, CSB_MAX, F0P], dt.float32, tag="G")
                    for c in range(csb):
                        nc.gpsimd.indirect_dma_start(
                            out=G[:, c, :], out_offset=None,
                            in_=xt_full[:],
                            in_offset=bass.IndirectOffsetOnAxis(
                                ap=idx_t[:, c:c + 1], axis=0),
                        )
                    for b in blks:
                        t0 = int(pl.col0[b]) - cstart
                        Tb = int(pl.T[b])
                        dv = dinv_all[:, b:b + 1]
                        acc = bp.tile([P, F0P], dt.float32, tag="acc")
                        nc.vector.tensor_reduce(
                            acc[:], G[:, t0:t0 + Tb, :].transpose([0, 2, 1]),
                            axis=mybir.AxisListType.X, op=mybir.AluOpType.add)
                        accs = bp.tile([P, F0P], dt.float32, tag="accs")
                        nc.vector.tensor_scalar_mul(accs[:], acc[:], dv)
                        tp1 = pp.tile([F0P, P], dt.float32, space="PSUM", tag="tp1")
                        nc.tensor.transpose(tp1[:], accs[:], ident[:])
                        tT = bp.tile([F0P, P], dt.float32, tag="tT")
                        nc.scalar.copy(tT[:], tp1[:])
                        h1p = pp.tile([P, F1], dt.float32, space="PSUM", tag="h1p")
                        nc.tensor.matmul(h1p[:], lhsT=tT[:], rhs=w1_s[:],
                                         start=True, stop=True)
                        h1b = bp.tile([P, F1], dt.float32, tag="h1b")
                        nc.vector.tensor_tensor(h1b[:], h1p[:], b1_bc[:],
                                                op=mybir.AluOpType.add)
                        h1t = bp.tile([P, F1], dt.float32, tag="h1t")
                        nc.scalar.activation(h1t[:], h1b[:],
                                             mybir.ActivationFunctionType.Tanh)
                        h1s = bp.tile([P, F1], dt.float32, tag="h1s")
                        nc.vector.tensor_scalar_mul(h1s[:], h1t[:], dv)
                        tp2 = pp.tile([F1, P], dt.float32, space="PSUM", tag="tp2")
                        nc.tensor.transpose(tp2[:], h1s[:], ident[:])
                        t2 = bp.tile([F1, P], dt.float32, tag="t2")
                        nc.scalar.copy(t2[:], tp2[:])
                        gp_ = pp.tile([P, F2], dt.float32, space="PSUM", tag="gp")
                        nc.tensor.matmul(gp_[:], lhsT=t2[:], rhs=w2_s[:],
                                         start=True, stop=True)
                        nc.vector.tensor_copy(gs_w[:, b, :], gp_[:])
                # write g~ shard in chunks
                for j0 in range(0, NB, CH):
                    nch = min(CH, NB - j0)
                    dstv = gt_shard[j0 * P:(j0 + nch) * P, :].rearrange(
                        "(j p) f -> p j f", p=P)
                    nc.sync.dma_start(dstv, gs_w[:, j0:j0 + nch, :])
            all_gather(gt_shard[:], gt_full[:])

            # ---------------- phase 4: layer 2 + head + pooling -------------
            with tc.tile_pool(name="g2", bufs=3) as gp2, \
                 tc.tile_pool(name="ix2", bufs=3) as ixp2, \
                 tc.tile_pool(name="blk2", bufs=4) as bp2, \
                 tc.tile_pool(name="ps2", bufs=2, space="PSUM") as pp2:
                for cstart, blks in pl.sbs:
                    csb = int(sum(pl.T[b] for b in blks))
                    idx_t = ixp2.tile([P, CSB_MAX], dt.int32, tag="idx2")
                    nc.sync.dma_start(idx_t[:, :csb], idx_all[:, cstart:cstart + csb])
                    G2 = gp2.tile([P, CSB_MAX, F2], dt.float32, tag="G2")
                    for c in range(csb):
                        nc.gpsimd.indirect_dma_start(
                            out=G2[:, c, :], out_offset=None,
                            in_=gt_full[:],
                            in_offset=bass.IndirectOffsetOnAxis(
                                ap=idx_t[:, c:c + 1], axis=0),
                        )
                    for b in blks:
                        t0 = int(pl.col0[b]) - cstart
                        Tb = int(pl.T[b])
                        dv = dinv_all[:, b:b + 1]
                        acc2 = bp2.tile([P, F2], dt.float32, tag="acc2")
                        nc.vector.tensor_reduce(
                            acc2[:], G2[:, t0:t0 + Tb, :].transpose([0, 2, 1]),
                            axis=mybir.AxisListType.X, op=mybir.AluOpType.add)
                        a2s = bp2.tile([P, F2], dt.float32, tag="a2s")
                        nc.vector.tensor_scalar_mul(a2s[:], acc2[:], dv)
                        a2b = bp2.tile([P, F2], dt.float32, tag="a2b")
                        nc.vector.tensor_tensor(a2b[:], a2s[:], b2_bc[:],
                                                op=mybir.AluOpType.add)
                        h2 = bp2.tile([P, F2], dt.float32, tag="h2")
                        nc.scalar.activation(h2[:], a2b[:],
                                             mybir.ActivationFunctionType.Tanh)
                        tp3 = pp2.tile([F2, P], dt.float32, space="PSUM", tag="tp3")
                        nc.tensor.transpose(tp3[:], h2[:], ident[:])
                        hT = bp2.tile([F2, P], dt.float32, tag="hT")
                        nc.scalar.copy(hT[:], tp3[:])
                        zp = pp2.tile([P, 1], dt.float32, space="PSUM", tag="zp")
                        nc.tensor.matmul(zp[:], lhsT=hT[:], rhs=wh_s[:],
                                         start=True, stop=True)
                        zs = bp2.tile([P, 1], dt.float32, tag="zs")
                        nc.vector.tensor_copy(zs[:], zp[:])
                        S = bp2.tile([P, GPC], dt.float32, tag="S")
                        nc.vector.tensor_scalar(
                            S[:], iota_f[:], batchl_all[:, b:b + 1], None,
                            op0=mybir.AluOpType.is_equal)
                        ppsum = pp2.tile([GPC, 1], dt.float32, space="PSUM", tag="pp")
                        nc.tensor.matmul(ppsum[:], lhsT=S[:], rhs=zs[:],
                                         start=True, stop=True)
                        nc.vector.tensor_tensor(util_acc[:], util_acc[:], ppsum[:],
                                                op=mybir.AluOpType.add)

            # ---------------- phase 5: pooling finalize + pair diffs --------
            with tc.tile_pool(name="p5", bufs=1) as p5:
                util0 = p5.tile([GPC, 1], dt.float32)
                nc.vector.tensor_scalar(
                    util0[:], util_acc[:], invcnt_s[:], bh_bc[:],
                    op0=mybir.AluOpType.mult, op1=mybir.AluOpType.add)
                util = p5.tile([GPC, 1], dt.float32)
                nc.vector.tensor_scalar_mul(util[:], util0[:], gmask_s[:])
                nc.sync.dma_start(xu_out[:], util[:])
                nc.sync.dma_start(util_sh[:], util[:])
                all_gather(util_sh[:], util_full[:])
                pa_t = p5.tile([pl.p_use, pl.tp], dt.int32)
                nc.sync.dma_start(pa_t[:], pair_a[:])
                pb_t = p5.tile([pl.p_use, pl.tp], dt.int32)
                nc.sync.dma_start(pb_t[:], pair_b[:])
                ua = p5.tile([pl.p_use, pl.tp, 1], dt.float32)
                ub = p5.tile([pl.p_use, pl.tp, 1], dt.float32)
                for t in range(pl.tp):
                    nc.gpsimd.indirect_dma_start(
                        out=ua[:, t, :], out_offset=None, in_=util_full[:],
                        in_offset=bass.IndirectOffsetOnAxis(ap=pa_t[:, t:t + 1], axis=0))
                    nc.gpsimd.indirect_dma_start(
                        out=ub[:, t, :], out_offset=None, in_=util_full[:],
                        in_offset=bass.IndirectOffsetOnAxis(ap=pb_t[:, t:t + 1], axis=0))
                d = p5.tile([pl.p_use, pl.tp], dt.float32)
                nc.vector.tensor_tensor(d[:], ub[:].squeeze(2), ua[:].squeeze(2),
                                        op=mybir.AluOpType.subtract)
                nc.sync.dma_start(
                    pairs_out[:].rearrange("(p t) o -> p (t o)", p=pl.p_use), d[:])

    nc.compile()
    return nc


# ---------------------------------------------------------------------------
# Entry points
# ---------------------------------------------------------------------------

def prepare(inputs, n_cores=N_CORES):
    """Build plan, program and per-core input maps from full inputs."""
    x = np.asarray(inputs["x"], dtype=np.float32)
    edge_index = np.asarray(inputs["edge_index"])
    batch = np.asarray(inputs["batch"])
    idx_a = np.asarray(inputs["idx_a"])
    idx_b = np.asarray(inputs["idx_b"])
    n_nodes = x.shape[0]
    n_graphs = int(max(batch.max(), idx_a.max(), idx_b.max())) + 1
    n_graphs = ((n_graphs + n_cores - 1) // n_cores) * n_cores
    n_pairs = idx_a.shape[0]

    pl = make_plan(edge_index, batch, n_nodes, n_graphs, n_cores, n_pairs,
                   idx_a, idx_b)

    W1 = np.asarray(inputs["W1"], dtype=np.float32)
    w1p = np.zeros((F0P, F1), dtype=np.float32)
    w1p[:W1.shape[0], :] = W1
    b1 = np.asarray(inputs["b1"], dtype=np.float32)[None, :]
    W2 = np.asarray(inputs["W2"], dtype=np.float32)
    b2 = np.asarray(inputs["b2"], dtype=np.float32)[None, :]
    Wf1T = np.ascontiguousarray(np.asarray(inputs["Wf1"], dtype=np.float32).T)
    Wf2 = np.asarray(inputs["Wf2"], dtype=np.float32).reshape(F2, 1)
    bf1 = np.asarray(inputs["bf1"], dtype=np.float32).reshape(F2, 1)
    bf2 = np.asarray(inputs["bf2"], dtype=np.float32).reshape(1, 1)

    in_maps = []
    for c in range(n_cores):
        xs = np.zeros((pl.NSH, F0P), dtype=np.float32)
        o = pl.order_c[c]
        xs[:len(o), :x.shape[1]] = x[o]
        in_maps.append({
            "x_shard": xs,
            "idx_all": pl.idx_all[c],
            "dinv_pb": pl.dinv_pb[c],
            "batchl_pb": pl.batchl_pb[c],
            "invcnt": pl.invcnt[c],
            "gmask": pl.gmask[c],
            "pair_a": pl.pair_a[c],
            "pair_b": pl.pair_b[c],
            "w1p": w1p, "b1d": b1, "w2d": W2, "b2d": b2,
            "wf1t": Wf1T, "wf2d": Wf2, "bf1d": bf1, "bf2d": bf2,
        })
    nc = build_program(pl)
    return nc, pl, in_maps


def assemble(pl, results):
    n_cores = pl.n_cores
    out = np.concatenate(
        [results[c]["pairs_out"].reshape(-1)[:pl.ppc] for c in range(n_cores)])
    xu = np.concatenate([results[c]["xu_out"] for c in range(n_cores)], axis=0)
    return out.astype(np.float32), xu.astype(np.float32)


def kernel(**inputs):
    nc, pl, in_maps = prepare(inputs, N_CORES)
    res = bass_utils.run_bass_kernel_spmd(nc, in_maps, core_ids=list(range(N_CORES)))
    return assemble(pl, res.results)
